# revision 47
# baseline (speedup 1.0000x reference)
"""Trainium2 Bass kernel for nn_DifferentiableSimulator.

Strategy (8 NeuronCores, B=8): one batch element per core, no collectives.

Host side (cheap, O(V+N)):
  - per-batch probe geometry: rotation, LUT bilinear interp (tiny)
  - per-batch voxel relevance sharding: keep voxels within CUT(7.5mm) +
    probe-radius of the shank axis segment.  Dropped voxels have weights
    < e^-14 relative to any weight that can influence an output pixel;
    empirically the output matches the dense reference to ~1e-3.
  - lattice factorization: the 1000 contacts are a rigid 10x10x10 grid,
    so in the rotated frame  d2[n,v] = (x_i-wx_v)^2 + (y_j-wy_v)^2 +
    (z_k-wz_v)^2  with w = R^T (v - grid_center).  The soft-match weight
    matrix factorizes as W[n,v] = Wxy[(ij),v] * Wz[k,v]: only 110 gaussian
    columns per voxel instead of 1000.  Host ships the voxel features
    (fp16 hi/lo pairs so the fp16 matmul is ~fp32-exact: fp16 products are
    exact in the fp32 PSUM accumulator) and the 138 lattice columns.
  - contacts are reindexed m = k*128 + (iy*10+ix)  (28 dummy xy slots per
    z-layer with weight 0) so the per-z-layer weighted sums land exactly
    in contact-chunk layout with no transposes.

Device side (per core), phase 1 -- soft PRF match, halves of the voxel
chunks: cross matmuls for a half land in one PSUM tile, ONE mega-exp
converts the half to fp16 gaussian weights, then per chunk a DVE op
forms WzE (fp16) and a single-pass fp16 matmul accumulates
B[128ij, 30] = sum_v Wxy^T (Wz*E) in fp32 PSUM.

Phase 2 -- separable splat (phos_sigma*SE < 0.46 for every reachable
ecc >= 0, so the max(.,1) clamp makes every phosphene sigma exactly
1 px; the 1/s scale drops out).  Per-contact centers via the hardware
Sin spline (sin table preloaded by a dummy op during phase 1, exp
table reloaded by a dummy right after).  Row/col gaussian arguments are
computed in TWO mega DVE/GpSimd ops per batch (broadcast APs over the
chunk axis amortize the ~160ns DVE instruction overhead), one mega-exp
per batch, then 20 fp16 matmuls accumulate the 256x256 map.

Normalize: row maxes on DVE, cross-partition max via the GpSimd
partition_all_reduce, reciprocal per partition, scale on DVE+ACT,
DMA out on two queues.
"""
import math
from contextlib import ExitStack

import numpy as np

import concourse.bass as bass
import concourse.bass_isa as bass_isa
import concourse.mybir as mybir
from concourse import tile
from concourse.bass_utils import run_bass_kernel_spmd

# ---- constants (must match the reference) ----
_CMAG_A = 0.75
_CMAG_B = 120.0
_CMAG_K = 17.3
_DEG2RAD = math.pi / 180.0
AMP = 100.0
_SPREAD = math.sqrt(AMP / 675.0)
VIEW_ANGLE = 90.0
MAP_SIZE = 256
SOFT_MATCH_SIGMA = 1.5

B = 8
NCC = 10                  # contact chunks = z-layers
NXY = 128                 # xy-lattice slots per layer (100 real + 28 dummy)
CUT = 7.5
XY_RAD = 1.8 * math.sqrt(2.0)
SE = MAP_SIZE / VIEW_ANGLE
EXP_SCALE = 2.0 / (2.0 * SOFT_MATCH_SIGMA ** 2)   # 2/4.5
NL = NXY + 10             # 138 lattice columns

f32 = mybir.dt.float32
f16 = mybir.dt.float16
i32 = mybir.dt.int32
AF = mybir.ActivationFunctionType
ALU = mybir.AluOpType
PI = math.pi


# ---------------------------------------------------------------- host prep
def _f16s(x):
    hi = np.float16(x)
    lo = np.float16(np.float32(x) - np.float32(hi))
    return hi, lo


def _f16_split(x):
    hi = x.astype(np.float16)
    lo = (x.astype(np.float32) - hi.astype(np.float32)).astype(np.float16)
    return hi.astype(np.float32), lo.astype(np.float32)


def _host_geometry(params, start_loc, surf_dist_lut, alpha_grid, beta_grid):
    params = params.astype(np.float64)
    alpha, beta, offset, shank = (params[:, 0], params[:, 1],
                                  params[:, 2], params[:, 3])
    a = alpha * _DEG2RAD
    b = beta * _DEG2RAD
    ca, sa = np.cos(a), np.sin(a)
    cb, sb = np.cos(b), np.sin(b)
    Bn = params.shape[0]
    Rx = np.zeros((Bn, 3, 3)); Ry = np.zeros((Bn, 3, 3))
    Rx[:, 0, 0] = 1; Rx[:, 1, 1] = ca; Rx[:, 1, 2] = -sa
    Rx[:, 2, 1] = sa; Rx[:, 2, 2] = ca
    Ry[:, 0, 0] = cb; Ry[:, 0, 2] = sb; Ry[:, 1, 1] = 1
    Ry[:, 2, 0] = -sb; Ry[:, 2, 2] = cb
    R = Rx @ Ry
    direction = np.einsum('bij,j->bi', R, np.array([0.0, 0.0, -1.0]))
    direction = direction / np.linalg.norm(direction, axis=-1, keepdims=True)
    lut = surf_dist_lut.astype(np.float64)
    na, nb = lut.shape
    ag, bg = alpha_grid.astype(np.float64), beta_grid.astype(np.float64)
    a_norm = 2.0 * (alpha - ag[0]) / (ag[-1] - ag[0] + 1e-08) - 1.0
    b_norm = 2.0 * (beta - bg[0]) / (bg[-1] - bg[0] + 1e-08) - 1.0
    ai = np.clip((a_norm + 1.0) * 0.5 * (na - 1), 0.0, na - 1.0)
    bi = np.clip((b_norm + 1.0) * 0.5 * (nb - 1), 0.0, nb - 1.0)
    a0 = np.clip(np.floor(ai), 0, na - 1).astype(np.int64)
    b0 = np.clip(np.floor(bi), 0, nb - 1).astype(np.int64)
    a1 = np.minimum(a0 + 1, na - 1)
    b1 = np.minimum(b0 + 1, nb - 1)
    fa = ai - a0
    fb = bi - b0
    v00 = lut[a0, b0]; v01 = lut[a0, b1]; v10 = lut[a1, b0]; v11 = lut[a1, b1]
    surf = (v00 * (1 - fa) * (1 - fb) + v01 * (1 - fa) * fb
            + v10 * fa * (1 - fb) + v11 * fa * fb)
    surf = np.maximum(surf, 1.0)
    penetration = surf - shank / 2.0 - offset
    grid_center = (start_loc.astype(np.float64)[None, :]
                   + direction * penetration[:, None])
    return grid_center, R, direction, shank


def _voxel_keep(v1_pos, grid_center, axis_dir, half_len):
    d = v1_pos.astype(np.float64) - grid_center[None, :]
    t = np.clip(d @ axis_dir, -half_len, half_len)
    dist = np.linalg.norm(d - t[:, None] * axis_dir[None, :], axis=1)
    return dist <= (CUT + XY_RAD + 0.5)


def _prep_core(gc_b, R_b, shank_b, logits_b, v1_pos_k, v1_prf_k, VP, VP0):
    """Per-core device input arrays for the lattice-factorized kernel."""
    Vk = v1_pos_k.shape[0]
    w = np.zeros((VP, 3))
    w[:Vk] = (v1_pos_k.astype(np.float64) - gc_b[None, :]) @ R_b
    wf = w.astype(np.float32)
    wh, wl = _f16_split(wf)
    bxy = (-0.5 * (w[:, 0] ** 2 + w[:, 1] ** 2)).astype(np.float32)
    bz = (-0.5 * w[:, 2] ** 2).astype(np.float32)
    bxy[Vk:] = -30000.0
    bz[Vk:] = -30000.0
    bxyh, bxyl = _f16_split(bxy)
    bzh, bzl = _f16_split(bz)
    onesv = np.ones(VP, np.float32)
    vt = np.stack([wh[:, 0], wh[:, 1], wl[:, 0], wl[:, 1], wh[:, 0],
                   wh[:, 1], onesv, onesv, bxyh, bxyl,
                   wh[:, 2], wl[:, 2], wh[:, 2], onesv, onesv, bzh, bzl],
                  axis=0).astype(np.float16)

    xs = np.arange(10) * 0.4 - 1.8
    zs = (np.linspace(0.0, 1.0, 10) - 0.5) * float(shank_b)
    cols = np.zeros((17, NXY + 10), np.float32)
    for ij in range(NXY):
        if ij < 100:
            iy, ix = ij // 10, ij % 10
            x, y = xs[ix], xs[iy]
            xh, xl = _f16s(x)
            yh, yl = _f16s(y)
            axyh, axyl = _f16s(-0.5 * (x * x + y * y))
            cols[0:10, ij] = [xh, yh, xh, yh, xl, yl, axyh, axyl, 1.0, 1.0]
        else:
            cols[6, ij] = -30000.0     # dummy xy slot -> Wxy = 0
            cols[8, ij] = 1.0
    for k in range(10):
        z = zs[k]
        zh, zl = _f16s(z)
        azh, azl = _f16s(-0.5 * z * z)
        cols[10:17, NXY + k] = [zh, zh, zl, azh, azl, 1.0, 1.0]
    rhs = cols.astype(np.float16)

    nch = VP // 128
    e3 = np.zeros((VP, 3), np.float32)
    e3[:Vk, 0] = v1_prf_k[:, 0]
    e3[:Vk, 1] = v1_prf_k[:, 1]
    e3[:Vk, 2] = 1.0
    e3t = np.ascontiguousarray(
        e3.reshape(nch, 128, 3).transpose(1, 0, 2).reshape(128, 3 * nch))

    lgt = np.full((NXY, NCC), -30.0, np.float32)
    iy, ix = np.divmod(np.arange(100), 10)
    for k in range(NCC):
        lgt[:100, k] = logits_b[iy * 100 + ix * 10 + k]
    lgt = 1.0 / (1.0 + np.exp(-lgt.astype(np.float64)))   # sigmoid on host
    vtc = np.ascontiguousarray(vt)
    return {"vt0": np.ascontiguousarray(vtc[:, :VP0]),
            "vt1": np.ascontiguousarray(vtc[:, VP0:]),
            "rhs": rhs, "e3": e3t,
            "lgt": np.ascontiguousarray(lgt.astype(np.float32)),
            "eye": np.eye(128, dtype=np.float16)}


# ------------------------------------------------------------- device kernel
def _split_multiwaits(nc):
    """This walrus build accepts at most ONE sync wait per instruction.
    Tile emits several.  Engine instruction streams execute in order, so
    moving all but one wait onto single-wait NoOps inserted just before
    the instruction preserves semantics exactly."""
    cnt = 0
    for fn in nc.m.functions:
        for blk in fn.blocks:
            out = []
            for inst in blk.instructions:
                si = inst.sync_info
                if si is not None and si.on_wait is not None \
                        and len(si.on_wait) > 1:
                    waits = list(si.on_wait)
                    for w in waits[:-1]:
                        cnt += 1
                        out.append(mybir.InstNoOp(
                            name=f"WSPLIT-{cnt}",
                            engine=inst.engine,
                            ins=[], outs=[],
                            sync_info=mybir.SyncInfo(on_wait=[w],
                                                     on_update=[]),
                        ))
                    inst.sync_info = mybir.SyncInfo(
                        on_wait=[waits[-1]], on_update=list(si.on_update))
                out.append(inst)
            blk.instructions = out
    return cnt


def _build_nc(VP):
    nch = VP // 128
    h0 = (nch + 1) // 2          # chunks in first half
    h1 = nch - h0
    VP0 = h0 * 128
    nc = bass.Bass()
    vt0_d = nc.dram_tensor("vt0", [17, VP0], f16, kind="ExternalInput")
    vt1_d = (nc.dram_tensor("vt1", [17, VP - VP0], f16, kind="ExternalInput")
             if h1 else None)
    rhs_d = nc.dram_tensor("rhs", [17, NL], f16, kind="ExternalInput")
    e3_d = nc.dram_tensor("e3", [128, 3 * nch], f32, kind="ExternalInput")
    lgt_d = nc.dram_tensor("lgt", [NXY, NCC], f32, kind="ExternalInput")
    eye_d = nc.dram_tensor("eye", [128, 128], f16, kind="ExternalInput")
    out_d = nc.dram_tensor("out", [MAP_SIZE, MAP_SIZE], f32,
                           kind="ExternalOutput")

    with ExitStack() as ctx:
        tc = ctx.enter_context(tile.TileContext(nc))
        constp = ctx.enter_context(tc.tile_pool(name="const", bufs=1))
        parm = ctx.enter_context(tc.tile_pool(name="parm", bufs=1))
        work = ctx.enter_context(tc.tile_pool(name="work", bufs=6))
        psB = ctx.enter_context(
            tc.tile_pool(name="psB", bufs=1, space=bass.MemorySpace.PSUM))

        # Warmups first (top scheduler priority): ACT table load + PE HAM
        # burst run during the sem-init + input-DMA window.
        scr = constp.tile([1, 1], f32, tag="scr", name="scr")
        nc.vector.memset(scr[:], 0.0)
        nc.scalar.activation(scr[:], scr[:], AF.Exp, bias=0.0, scale=1.0)
        wrm = constp.tile([128, 256], f16, tag="wrm", name="wrm")
        nc.vector.memset(wrm[:], 0.0)
        with tc.tile_pool(name="psWp", bufs=1,
                          space=bass.MemorySpace.PSUM) as psWp:
            wps = psWp.tile([128, 256], f32, tag="wps", name="wps")
            for _ in range(16):
                nc.tensor.matmul(wps[:], wrm[:, 0:128], wrm[:],
                                 start=True, stop=True, skip_group_check=True)
            # DVE + ACT warmup burst in the same dead window
            wrv = constp.tile([128, 2048], f32, tag="wrv", name="wrv")
            nc.vector.memset(wrv[:], 1.0)
            nc.vector.tensor_tensor(wrv[:], wrv[:], wrv[:], ALU.mult)
            nc.vector.tensor_tensor(wrv[:], wrv[:], wrv[:], ALU.mult)
            nc.scalar.activation(wrv[:, 0:1024], wrv[:, 0:1024], AF.Exp,
                                 bias=0.0, scale=0.0)

        # ---------------- input DMAs (4 queues) ----------------
        rhs_t = constp.tile([17, NL], f16, tag="rhs", name="rhs")
        nc.sync.dma_start(rhs_t[:], rhs_d[:])
        vt_t0 = constp.tile([17, VP0], f16, tag="vt0", name="vt0")
        nc.sync.dma_start(vt_t0[:], vt0_d[:])
        e3_t = constp.tile([128, 3 * nch], f32, tag="e3", name="e3")
        nc.sync.dma_start(e3_t[:], e3_d[:])
        if h1:
            vt_t1 = constp.tile([17, VP - VP0], f16, tag="vt1", name="vt1")
            nc.scalar.dma_start(vt_t1[:], vt1_d[:])
        eye_t = constp.tile([128, 128], f16, tag="eye", name="eye")
        nc.gpsimd.dma_start(eye_t[:], eye_d[:])
        ones16 = constp.tile([1, 128], f16, tag="ones16", name="ones16")
        nc.vector.memset(ones16[:], 1.0)
        lg_t = constp.tile([NXY, NCC], f32, tag="lgt", name="lgt")
        nc.scalar.dma_start(lg_t[:], lgt_d[:])

        # Window: every phosphene center is within |c-128| <= 12*SE+eps
        # = 34.2 px and sigma == 1 px, so the map is (sub-1e-6) zero
        # outside the centered 128x128 window [64,192).  Compute factors,
        # matmuls, and normalization on the window only; pre-write the
        # zero border during the input-DMA dead time.
        WIN, WOFF = 96, 80
        ii_t = constp.tile([128, WIN], i32, tag="ii", name="ii")
        nc.gpsimd.iota(ii_t[:], pattern=[[1, WIN]], base=0,
                       channel_multiplier=0)
        iof = constp.tile([128, WIN], f32, tag="iof", name="iof")
        nc.vector.tensor_copy(iof[:], ii_t[:])

        zt = constp.tile([128, MAP_SIZE], f32, tag="zt", name="zt")
        nc.vector.memset(zt[:], 0.0)
        nc.sync.dma_start(out_d[0:WOFF, :], zt[0:WOFF, :])
        nc.sync.dma_start(out_d[WOFF + WIN:MAP_SIZE, :], zt[0:WOFF, :])
        nc.gpsimd.dma_start(out_d[WOFF:WOFF + WIN, 0:WOFF],
                            zt[0:WIN, 0:WOFF])
        nc.gpsimd.dma_start(out_d[WOFF:WOFF + WIN, WOFF + WIN:MAP_SIZE],
                            zt[0:WIN, 0:WOFF])

        pb = lg_t        # sigmoid(logits), computed on host

        # ---------------- phase 1: factorized soft match ----------------
        B_ps = psB.tile([128, 3 * NCC], f32, tag="B", name="B")
        halves = [(0, h0)] + ([(h0, h1)] if h1 else [])
        with tc.tile_pool(name="psW", bufs=2,
                          space=bass.MemorySpace.PSUM) as psW:
            wx_list = []
            for hi_, (c0, hn) in enumerate(halves):
                vt_h = vt_t0 if hi_ == 0 else vt_t1
                ct = psW.tile([128, hn * NL], f32, tag=f"cross{hi_}",
                              name=f"cross{hi_}")
                wx = work.tile([128, hn * NL], f16, tag=f"wx{hi_}",
                               name=f"wx{hi_}")
                for j in range(hn):
                    nc.tensor.matmul(ct[:, j * NL:(j + 1) * NL],
                                     vt_h[:, j * 128:(j + 1) * 128],
                                     rhs_t[:], start=True, stop=True)
                nc.scalar.activation(wx[:], ct[:], AF.Exp,
                                     bias=0.0, scale=EXP_SCALE)
                wx_list.append(wx)
            for hi_, (c0, hn) in enumerate(halves):
                wx = wx_list[hi_]
                for j in range(hn):
                    c = c0 + j
                    wze = work.tile([128, 3 * NCC], f16, tag="wze",
                                    name=f"wze{c}")
                    e3b = e3_t[:, 3 * c:3 * c + 3] \
                        .rearrange("p (one f) -> p one f", one=1) \
                        .broadcast_to([128, NCC, 3])
                    wzb = wx[:, j * NL + NXY:(j + 1) * NL] \
                        .rearrange("p (k one) -> p k one", one=1) \
                        .broadcast_to([128, NCC, 3])
                    nc.vector.tensor_tensor(
                        wze[:].rearrange("p (k f) -> p k f", f=3),
                        e3b, wzb, ALU.mult)
                    nc.tensor.matmul(B_ps[:], wx[:, j * NL:j * NL + NXY],
                                     wze[:], start=(c == 0),
                                     stop=(c == nch - 1))

        # dummy Sin reading the last phase-1 exp output: anchors the
        # sin-table load right after the phase-1 exps in the ACT stream,
        # so it runs during the B-accumulate window.
        wx_last = wx_list[-1]
        lo = (halves[-1][1] - 1) * NL
        nc.scalar.activation(scr[:], wx_last[0:1, lo:lo + 1], AF.Sin)

        bs3 = B_ps[:].rearrange("p (k f) -> p k f", f=3)

        with tc.tile_pool(name="psM", bufs=1,
                          space=bass.MemorySpace.PSUM) as psM:
            def pt(tag):
                return parm.tile([128, NCC], f32, tag=tag, name=tag)

            # ---------------- per-contact params ----------------
            # phos_size == 1 always (max KSIG/|m| = 0.46 < 1 for ecc>=0),
            # so sr == 1 and the whole magnification chain drops out.
            t0 = pt("t0")
            nc.vector.tensor_scalar_add(t0[:], bs3[:, :, 2], 1e-8)
            rws = pt("rws"); nc.vector.reciprocal(rws[:], t0[:])
            # pe2 = [pol | ecc] in one op
            pe2 = parm.tile([128, 2 * NCC], f32, tag="pe2", name="pe2")
            rwsb = rws[:].rearrange("p (one k) -> p one k", one=1) \
                .broadcast_to([128, 2, NCC])
            bpol = B_ps[:].rearrange("p (k f) -> p f k", f=3)[:, 0:2, :]
            nc.vector.tensor_tensor(
                pe2[:].rearrange("p (f k) -> p f k", f=2),
                bpol, rwsb, ALU.mult)
            pol = pe2[:, 0:NCC]
            ecc = pe2[:, NCC:2 * NCC]

            # t20 = [pi/2 - |theta| | theta]; ACT Sin gives [cos | sin].
            t20 = parm.tile([128, 2 * NCC], f32, tag="t20", name="t20")
            nc.vector.tensor_scalar(t20[:, NCC:2 * NCC], pol, _DEG2RAD, -PI,
                                    ALU.mult, ALU.add)
            nc.vector.tensor_scalar(t20[:, NCC:2 * NCC], t20[:, NCC:2 * NCC],
                                    PI, -PI, ALU.min, ALU.max)
            ya = pt("ya")
            nc.scalar.activation(ya[:], t20[:, NCC:2 * NCC], AF.Abs)
            nc.vector.tensor_scalar(t20[:, 0:NCC], ya[:], -1.0, PI / 2.0,
                                    ALU.mult, ALU.add)
            sc20 = parm.tile([128, 2 * NCC], f32, tag="sc20", name="sc20")
            nc.scalar.activation(sc20[:], t20[:], AF.Sin)
            # dummy exp reading the Sin output: anchors the exp-table
            # reload right after the Sin, ahead of the phase-2 exps.
            nc.scalar.activation(scr[:], sc20[0:1, 0:1], AF.Exp,
                                 bias=0.0, scale=1.0)
            cs = sc20[:, 0:NCC]
            sn = sc20[:, NCC:2 * NCC]

            # t12 = [ecc*cos | ecc*sin] in one op
            t12 = parm.tile([128, 2 * NCC], f32, tag="t12", name="t12")
            eccb = pe2[:].rearrange("p (f k) -> p f k", f=2)[:, 1:2, :] \
                .broadcast_to([128, 2, NCC])
            nc.vector.tensor_tensor(
                t12[:].rearrange("p (f k) -> p f k", f=2),
                eccb, sc20[:].rearrange("p (f k) -> p f k", f=2), ALU.mult)
            sby = pt("sby")
            nc.vector.tensor_scalar(sby[:], t12[:, 0:NCC], -SE,
                                    -127.0 + WOFF, ALU.mult, ALU.add)
            sbx = pt("sbx")
            nc.vector.tensor_scalar(sbx[:], t12[:, NCC:2 * NCC], SE,
                                    -128.0 + WOFF, ALU.mult, ALU.add)

            # val/wc only feed the yy stage -- emit after the sby/sbx
            # critical chain so the scheduler keeps them off it
            val = pt("val")
            nc.vector.tensor_scalar_min(val[:], bs3[:, :, 2], 1.0)
            wc = parm.tile([128, NCC], f16, tag="wc", name="wc")
            nc.vector.tensor_mul(wc[:], pb[:], val[:])



            # ---------------- phase 2: separable splat ----------------
            # All factors computed on the 128-wide center window.
            # Squares split: ACT Square (bias per partition) for batch-0
            # sides and cols {2,3}; DVE (xs,sq) pair-ops for the rest.
            # GpSimd does NO phase-2 elementwise: concurrent DVE+GpSimd
            # SBUF traffic slows DVE ~3.5x (measured).
            mp = psM.tile([WIN, WIN], f32, tag="map", name="map")
            BATCHES = [(0, 2), (2, 4), (6, 4)]
            ACT_ROWS = {0, 1}
            ACT_COLS = {0, 1, 2, 3, 8, 9}
            sq_tiles = {}
            for b0, BN in BATCHES:
                W = BN * WIN
                sqb = work.tile([128, 2 * W], f16, tag="sqb",
                                name=f"sqb{b0}")
                sq_tiles[b0] = sqb

                def emit_sides(base, sbv, members):
                    q = 0
                    while q < BN:
                        c = b0 + q
                        if c in members:
                            nc.scalar.activation(
                                sqb[:, base + q * WIN:base + (q + 1) * WIN],
                                iof[:], AF.Square, bias=sbv[:, c:c + 1],
                                scale=1.0)
                            q += 1
                        else:
                            q1 = q
                            while q1 < BN and (b0 + q1) not in members:
                                q1 += 1
                            gn = q1 - q
                            xsp = work.tile([128, gn * WIN], f16,
                                            tag="xsp",
                                            name=f"xsp{base}_{b0}_{q}")
                            iofp = iof[:].rearrange(
                                "p (one n) -> p one n", one=1) \
                                .broadcast_to([128, gn, WIN])
                            sbp = sbv[:, b0 + q:b0 + q + gn] \
                                .rearrange("p (k one) -> p k one", one=1) \
                                .broadcast_to([128, gn, WIN])
                            nc.vector.tensor_tensor(
                                xsp[:].rearrange("p (k n) -> p k n",
                                                 n=WIN),
                                iofp, sbp, ALU.add)
                            nc.vector.tensor_tensor(
                                sqb[:, base + q * WIN:
                                    base + (q + gn) * WIN],
                                xsp[:], xsp[:], ALU.mult)
                            q = q1
                emit_sides(0, sby, ACT_ROWS)
                emit_sides(W, sbx, ACT_COLS)
            for b0, BN in BATCHES:
                W = BN * WIN
                sqb = sq_tiles[b0]
                xy8 = work.tile([128, 2 * W], f16, tag="xy8",
                                name=f"xy8{b0}")
                nc.scalar.activation(xy8[:], sqb[:], AF.Exp,
                                     bias=0.0, scale=-1.0)
                yyb = work.tile([128, W], f16, tag="yyb", name=f"yyb{b0}")
                wcb = wc[:, b0:b0 + BN] \
                    .rearrange("p (k one) -> p k one", one=1) \
                    .broadcast_to([128, BN, WIN])
                # high priority: once the batch's exp lands, the yy op
                # preempts remaining square work on DVE so the matmuls
                # can start
                with tc.high_priority():
                    nc.vector.tensor_tensor(
                        yyb[:].rearrange("p (k n) -> p k n", n=WIN),
                        xy8[:, 0:W].rearrange("p (k n) -> p k n", n=WIN),
                        wcb, ALU.mult)
                for q in range(BN):
                    c = b0 + q
                    yy = yyb[:, q * WIN:(q + 1) * WIN]
                    xx = xy8[:, W + q * WIN:W + (q + 1) * WIN]
                    nc.tensor.matmul(mp[:], yy, xx,
                                     start=(c == 0), stop=(c == NCC - 1))

            # ---------------- normalize + store ----------------
            # mx in f16: the max of f16-rounded row-maxes feeds an exact
            # f16 transpose (eye is 0/1) and an exact f16 broadcast
            # matmul -- +-2.4e-4 on the normalization only.
            mx = parm.tile([WIN, 1], f16, tag="mx", name="mx")
            nc.vector.reduce_max(mx[:], mp[:], axis=mybir.AxisListType.X)
            mt = psM.tile([1, WIN], f16, tag="mt", name="mt")
            nc.tensor.transpose(mt[:], mx[:], eye_t[0:WIN, 0:WIN])
            gm = parm.tile([1, 1], f16, tag="gm", name="gm")
            nc.vector.reduce_max(gm[:], mt[:], axis=mybir.AxisListType.X)
            gb = psM.tile([WIN, 1], f32, tag="gb", name="gb")
            nc.tensor.matmul(gb[:], ones16[:, 0:WIN], gm[:],
                             start=True, stop=True)
            ge = parm.tile([WIN, 1], f32, tag="ge", name="ge")
            nc.vector.tensor_scalar_add(ge[:], gb[:], 1e-8)
            gs = parm.tile([WIN, 1], f32, tag="gs", name="gs")
            nc.vector.reciprocal(gs[:], ge[:])

            o0 = work.tile([WIN, WIN], f32, tag="o0", name="o0")
            nc.vector.tensor_scalar_mul(o0[0:64, :], mp[0:64, :],
                                        gs[0:64, :])
            nc.sync.dma_start(out_d[WOFF:WOFF + 64, WOFF:WOFF + WIN],
                              o0[0:64, :])
            nc.scalar.activation(o0[64:WIN, :], mp[64:WIN, :], AF.Copy,
                                 scale=gs[64:WIN, :])
            nc.scalar.dma_start(
                out_d[WOFF + 64:WOFF + WIN, WOFF:WOFF + WIN],
                o0[64:WIN, :])
    return nc


# ----------------------------------------------------------------- entry
def _run(inputs, trace=False):
    params = np.asarray(inputs["params"], np.float32)
    logits = np.asarray(inputs["electrode_logits"], np.float32)
    v1_pos = np.asarray(inputs["v1_pos"], np.float32)
    v1_prf = np.asarray(inputs["v1_prf"], np.float32)
    start_loc = np.asarray(inputs["start_loc"], np.float32)
    surf_dist_lut = np.asarray(inputs["surf_dist_lut"], np.float32)
    alpha_grid = np.asarray(inputs["alpha_grid"], np.float32)
    beta_grid = np.asarray(inputs["beta_grid"], np.float32)

    gc, R, direction, shank = _host_geometry(
        params, start_loc, surf_dist_lut, alpha_grid, beta_grid)
    keeps = [_voxel_keep(v1_pos, gc[b], R[b, :, 2], shank[b] / 2.0)
             for b in range(B)]
    nkeep = max(int(k.sum()) for k in keeps)
    VP = max(256, ((nkeep + 127) // 128) * 128)
    nch = VP // 128
    VP0 = ((nch + 1) // 2) * 128

    in_maps = []
    for b in range(B):
        k = keeps[b]
        in_maps.append(_prep_core(gc[b], R[b], shank[b], logits[b],
                                  v1_pos[k], v1_prf[k], VP, VP0))
    nc = _build_nc(VP)
    _split_multiwaits(nc)
    res = run_bass_kernel_spmd(nc, in_maps, list(range(B)), trace=trace)
    out = np.stack([res.results[i]["out"] for i in range(B)])
    return out[:, None, :, :].astype(np.float32), res


def kernel(**inputs) -> np.ndarray:
    out, _ = _run(inputs, trace=False)
    return out


# revision 48
# speedup vs baseline: 1.1364x; 1.1364x over previous
"""Trainium2 Bass kernel for nn_DifferentiableSimulator.

Strategy (8 NeuronCores, B=8): one batch element per core, no collectives.

Host side (cheap, O(V+N)):
  - per-batch probe geometry: rotation, LUT bilinear interp (tiny)
  - per-batch voxel relevance sharding: keep voxels within CUT(7.5mm) +
    probe-radius of the shank axis segment.  Dropped voxels have weights
    < e^-14 relative to any weight that can influence an output pixel;
    empirically the output matches the dense reference to ~1e-3.
  - lattice factorization: the 1000 contacts are a rigid 10x10x10 grid,
    so in the rotated frame  d2[n,v] = (x_i-wx_v)^2 + (y_j-wy_v)^2 +
    (z_k-wz_v)^2  with w = R^T (v - grid_center).  The soft-match weight
    matrix factorizes as W[n,v] = Wxy[(ij),v] * Wz[k,v]: only 110 gaussian
    columns per voxel instead of 1000.  Host ships the voxel features
    (fp16 hi/lo pairs so the fp16 matmul is ~fp32-exact: fp16 products are
    exact in the fp32 PSUM accumulator) and the 138 lattice columns.
  - contacts are reindexed m = k*128 + (iy*10+ix)  (28 dummy xy slots per
    z-layer with weight 0) so the per-z-layer weighted sums land exactly
    in contact-chunk layout with no transposes.

Device side (per core), phase 1 -- soft PRF match, halves of the voxel
chunks: cross matmuls for a half land in one PSUM tile, ONE mega-exp
converts the half to fp16 gaussian weights, then per chunk a DVE op
forms WzE (fp16) and a single-pass fp16 matmul accumulates
B[128ij, 30] = sum_v Wxy^T (Wz*E) in fp32 PSUM.

Phase 2 -- separable splat (phos_sigma*SE < 0.46 for every reachable
ecc >= 0, so the max(.,1) clamp makes every phosphene sigma exactly
1 px; the 1/s scale drops out).  Per-contact centers via the hardware
Sin spline (sin table preloaded by a dummy op during phase 1, exp
table reloaded by a dummy right after).  Row/col gaussian arguments are
computed in TWO mega DVE/GpSimd ops per batch (broadcast APs over the
chunk axis amortize the ~160ns DVE instruction overhead), one mega-exp
per batch, then 20 fp16 matmuls accumulate the 256x256 map.

Normalize: row maxes on DVE, cross-partition max via the GpSimd
partition_all_reduce, reciprocal per partition, scale on DVE+ACT,
DMA out on two queues.
"""
import math
from contextlib import ExitStack

import numpy as np

import concourse.bass as bass
import concourse.bass_isa as bass_isa
import concourse.mybir as mybir
from concourse import tile
from concourse.bass_utils import run_bass_kernel_spmd

# ---- constants (must match the reference) ----
_CMAG_A = 0.75
_CMAG_B = 120.0
_CMAG_K = 17.3
_DEG2RAD = math.pi / 180.0
AMP = 100.0
_SPREAD = math.sqrt(AMP / 675.0)
VIEW_ANGLE = 90.0
MAP_SIZE = 256
SOFT_MATCH_SIGMA = 1.5

B = 8
NCC = 10                  # contact chunks = z-layers
NXY = 128                 # xy-lattice slots per layer (100 real + 28 dummy)
CUT = 7.5
XY_RAD = 1.8 * math.sqrt(2.0)
SE = MAP_SIZE / VIEW_ANGLE
EXP_SCALE = 2.0 / (2.0 * SOFT_MATCH_SIGMA ** 2)   # 2/4.5
NL = NXY + 10             # 138 lattice columns

f32 = mybir.dt.float32
f16 = mybir.dt.float16
i32 = mybir.dt.int32
AF = mybir.ActivationFunctionType
ALU = mybir.AluOpType
PI = math.pi


# ---------------------------------------------------------------- host prep
def _f16s(x):
    hi = np.float16(x)
    lo = np.float16(np.float32(x) - np.float32(hi))
    return hi, lo


def _f16_split(x):
    hi = x.astype(np.float16)
    lo = (x.astype(np.float32) - hi.astype(np.float32)).astype(np.float16)
    return hi.astype(np.float32), lo.astype(np.float32)


def _host_geometry(params, start_loc, surf_dist_lut, alpha_grid, beta_grid):
    params = params.astype(np.float64)
    alpha, beta, offset, shank = (params[:, 0], params[:, 1],
                                  params[:, 2], params[:, 3])
    a = alpha * _DEG2RAD
    b = beta * _DEG2RAD
    ca, sa = np.cos(a), np.sin(a)
    cb, sb = np.cos(b), np.sin(b)
    Bn = params.shape[0]
    Rx = np.zeros((Bn, 3, 3)); Ry = np.zeros((Bn, 3, 3))
    Rx[:, 0, 0] = 1; Rx[:, 1, 1] = ca; Rx[:, 1, 2] = -sa
    Rx[:, 2, 1] = sa; Rx[:, 2, 2] = ca
    Ry[:, 0, 0] = cb; Ry[:, 0, 2] = sb; Ry[:, 1, 1] = 1
    Ry[:, 2, 0] = -sb; Ry[:, 2, 2] = cb
    R = Rx @ Ry
    direction = np.einsum('bij,j->bi', R, np.array([0.0, 0.0, -1.0]))
    direction = direction / np.linalg.norm(direction, axis=-1, keepdims=True)
    lut = surf_dist_lut.astype(np.float64)
    na, nb = lut.shape
    ag, bg = alpha_grid.astype(np.float64), beta_grid.astype(np.float64)
    a_norm = 2.0 * (alpha - ag[0]) / (ag[-1] - ag[0] + 1e-08) - 1.0
    b_norm = 2.0 * (beta - bg[0]) / (bg[-1] - bg[0] + 1e-08) - 1.0
    ai = np.clip((a_norm + 1.0) * 0.5 * (na - 1), 0.0, na - 1.0)
    bi = np.clip((b_norm + 1.0) * 0.5 * (nb - 1), 0.0, nb - 1.0)
    a0 = np.clip(np.floor(ai), 0, na - 1).astype(np.int64)
    b0 = np.clip(np.floor(bi), 0, nb - 1).astype(np.int64)
    a1 = np.minimum(a0 + 1, na - 1)
    b1 = np.minimum(b0 + 1, nb - 1)
    fa = ai - a0
    fb = bi - b0
    v00 = lut[a0, b0]; v01 = lut[a0, b1]; v10 = lut[a1, b0]; v11 = lut[a1, b1]
    surf = (v00 * (1 - fa) * (1 - fb) + v01 * (1 - fa) * fb
            + v10 * fa * (1 - fb) + v11 * fa * fb)
    surf = np.maximum(surf, 1.0)
    penetration = surf - shank / 2.0 - offset
    grid_center = (start_loc.astype(np.float64)[None, :]
                   + direction * penetration[:, None])
    return grid_center, R, direction, shank


def _voxel_keep(v1_pos, grid_center, axis_dir, half_len):
    d = v1_pos.astype(np.float64) - grid_center[None, :]
    t = np.clip(d @ axis_dir, -half_len, half_len)
    dist = np.linalg.norm(d - t[:, None] * axis_dir[None, :], axis=1)
    return dist <= (CUT + XY_RAD + 0.5)


def _prep_core(gc_b, R_b, shank_b, logits_b, v1_pos_k, v1_prf_k, VP, VP0):
    """Per-core device input arrays for the lattice-factorized kernel."""
    Vk = v1_pos_k.shape[0]
    w = np.zeros((VP, 3))
    w[:Vk] = (v1_pos_k.astype(np.float64) - gc_b[None, :]) @ R_b
    wf = w.astype(np.float32)
    wh, wl = _f16_split(wf)
    bxy = (-0.5 * (w[:, 0] ** 2 + w[:, 1] ** 2)).astype(np.float32)
    bz = (-0.5 * w[:, 2] ** 2).astype(np.float32)
    bxy[Vk:] = -30000.0
    bz[Vk:] = -30000.0
    bxyh, bxyl = _f16_split(bxy)
    bzh, bzl = _f16_split(bz)
    onesv = np.ones(VP, np.float32)
    vt = np.stack([wh[:, 0], wh[:, 1], wl[:, 0], wl[:, 1], wh[:, 0],
                   wh[:, 1], onesv, onesv, bxyh, bxyl,
                   wh[:, 2], wl[:, 2], wh[:, 2], onesv, onesv, bzh, bzl],
                  axis=0).astype(np.float16)

    xs = np.arange(10) * 0.4 - 1.8
    zs = (np.linspace(0.0, 1.0, 10) - 0.5) * float(shank_b)
    cols = np.zeros((17, NXY + 10), np.float32)
    for ij in range(NXY):
        if ij < 100:
            iy, ix = ij // 10, ij % 10
            x, y = xs[ix], xs[iy]
            xh, xl = _f16s(x)
            yh, yl = _f16s(y)
            axyh, axyl = _f16s(-0.5 * (x * x + y * y))
            cols[0:10, ij] = [xh, yh, xh, yh, xl, yl, axyh, axyl, 1.0, 1.0]
        else:
            cols[6, ij] = -30000.0     # dummy xy slot -> Wxy = 0
            cols[8, ij] = 1.0
    for k in range(10):
        z = zs[k]
        zh, zl = _f16s(z)
        azh, azl = _f16s(-0.5 * z * z)
        cols[10:17, NXY + k] = [zh, zh, zl, azh, azl, 1.0, 1.0]
    rhs = cols.astype(np.float16)

    nch = VP // 128
    e3 = np.zeros((VP, 3), np.float32)
    e3[:Vk, 0] = v1_prf_k[:, 0]
    e3[:Vk, 1] = v1_prf_k[:, 1]
    e3[:Vk, 2] = 1.0
    e3t = np.ascontiguousarray(
        e3.reshape(nch, 128, 3).transpose(1, 0, 2).reshape(128, 3 * nch))

    lgt = np.full((NXY, NCC), -30.0, np.float32)
    iy, ix = np.divmod(np.arange(100), 10)
    for k in range(NCC):
        lgt[:100, k] = logits_b[iy * 100 + ix * 10 + k]
    lgt = 1.0 / (1.0 + np.exp(-lgt.astype(np.float64)))   # sigmoid on host
    vtc = np.ascontiguousarray(vt)
    return {"vt0": np.ascontiguousarray(vtc[:, :VP0]),
            "vt1": np.ascontiguousarray(vtc[:, VP0:]),
            "rhs": rhs, "e3": e3t,
            "lgt": np.ascontiguousarray(lgt.astype(np.float32)),
            "eye": np.eye(128, dtype=np.float16)}


# ------------------------------------------------------------- device kernel
def _split_multiwaits(nc):
    """This walrus build accepts at most ONE sync wait per instruction.
    Tile emits several.  Engine instruction streams execute in order, so
    moving all but one wait onto single-wait NoOps inserted just before
    the instruction preserves semantics exactly."""
    cnt = 0
    for fn in nc.m.functions:
        for blk in fn.blocks:
            out = []
            for inst in blk.instructions:
                si = inst.sync_info
                if si is not None and si.on_wait is not None \
                        and len(si.on_wait) > 1:
                    waits = list(si.on_wait)
                    for w in waits[:-1]:
                        cnt += 1
                        out.append(mybir.InstNoOp(
                            name=f"WSPLIT-{cnt}",
                            engine=inst.engine,
                            ins=[], outs=[],
                            sync_info=mybir.SyncInfo(on_wait=[w],
                                                     on_update=[]),
                        ))
                    inst.sync_info = mybir.SyncInfo(
                        on_wait=[waits[-1]], on_update=list(si.on_update))
                out.append(inst)
            blk.instructions = out
    return cnt


def _build_nc(VP):
    nch = VP // 128
    h0 = (nch + 1) // 2          # chunks in first half
    h1 = nch - h0
    VP0 = h0 * 128
    nc = bass.Bass()
    vt0_d = nc.dram_tensor("vt0", [17, VP0], f16, kind="ExternalInput")
    vt1_d = (nc.dram_tensor("vt1", [17, VP - VP0], f16, kind="ExternalInput")
             if h1 else None)
    rhs_d = nc.dram_tensor("rhs", [17, NL], f16, kind="ExternalInput")
    e3_d = nc.dram_tensor("e3", [128, 3 * nch], f32, kind="ExternalInput")
    lgt_d = nc.dram_tensor("lgt", [NXY, NCC], f32, kind="ExternalInput")
    eye_d = nc.dram_tensor("eye", [128, 128], f16, kind="ExternalInput")
    out_d = nc.dram_tensor("out", [MAP_SIZE, MAP_SIZE], f32,
                           kind="ExternalOutput")

    with ExitStack() as ctx:
        tc = ctx.enter_context(tile.TileContext(nc))
        constp = ctx.enter_context(tc.tile_pool(name="const", bufs=1))
        parm = ctx.enter_context(tc.tile_pool(name="parm", bufs=1))
        work = ctx.enter_context(tc.tile_pool(name="work", bufs=6))
        psB = ctx.enter_context(
            tc.tile_pool(name="psB", bufs=1, space=bass.MemorySpace.PSUM))

        # Warmups first (top scheduler priority): ACT table load + PE HAM
        # burst run during the sem-init + input-DMA window.
        scr = constp.tile([1, 1], f32, tag="scr", name="scr")
        nc.vector.memset(scr[:], 0.0)
        nc.scalar.activation(scr[:], scr[:], AF.Exp, bias=0.0, scale=1.0)
        wrm = constp.tile([128, 256], f16, tag="wrm", name="wrm")
        nc.vector.memset(wrm[:], 0.0)
        with tc.tile_pool(name="psWp", bufs=1,
                          space=bass.MemorySpace.PSUM) as psWp:
            wps = psWp.tile([128, 256], f32, tag="wps", name="wps")
            for _ in range(12):
                nc.tensor.matmul(wps[:], wrm[:, 0:128], wrm[:],
                                 start=True, stop=True, skip_group_check=True)

        # ---------------- input DMAs (4 queues) ----------------
        rhs_t = constp.tile([17, NL], f16, tag="rhs", name="rhs")
        nc.sync.dma_start(rhs_t[:], rhs_d[:])
        vt_t0 = constp.tile([17, VP0], f16, tag="vt0", name="vt0")
        nc.sync.dma_start(vt_t0[:], vt0_d[:])
        e3_t = constp.tile([128, 3 * nch], f32, tag="e3", name="e3")
        nc.sync.dma_start(e3_t[:], e3_d[:])
        if h1:
            vt_t1 = constp.tile([17, VP - VP0], f16, tag="vt1", name="vt1")
            nc.scalar.dma_start(vt_t1[:], vt1_d[:])
        eye_t = constp.tile([128, 128], f16, tag="eye", name="eye")
        nc.gpsimd.dma_start(eye_t[:], eye_d[:])
        ones16 = constp.tile([1, 128], f16, tag="ones16", name="ones16")
        nc.vector.memset(ones16[:], 1.0)
        lg_t = constp.tile([NXY, NCC], f32, tag="lgt", name="lgt")
        nc.scalar.dma_start(lg_t[:], lgt_d[:])

        # Window: every phosphene center is within |c-128| <= 12*SE+eps
        # = 34.2 px and sigma == 1 px, so the map is (sub-1e-6) zero
        # outside the centered 128x128 window [64,192).  Compute factors,
        # matmuls, and normalization on the window only; pre-write the
        # zero border during the input-DMA dead time.
        WIN, WOFF = 96, 80
        ii_t = constp.tile([128, WIN], i32, tag="ii", name="ii")
        nc.gpsimd.iota(ii_t[:], pattern=[[1, WIN]], base=0,
                       channel_multiplier=0)
        iof = constp.tile([128, WIN], f32, tag="iof", name="iof")
        nc.vector.tensor_copy(iof[:], ii_t[:])

        zt = constp.tile([128, MAP_SIZE], f32, tag="zt", name="zt")
        nc.vector.memset(zt[:], 0.0)
        nc.sync.dma_start(out_d[0:WOFF, :], zt[0:WOFF, :])
        nc.sync.dma_start(out_d[WOFF + WIN:MAP_SIZE, :], zt[0:WOFF, :])
        nc.gpsimd.dma_start(out_d[WOFF:WOFF + WIN, 0:WOFF],
                            zt[0:WIN, 0:WOFF])
        nc.gpsimd.dma_start(out_d[WOFF:WOFF + WIN, WOFF + WIN:MAP_SIZE],
                            zt[0:WIN, 0:WOFF])

        pb = lg_t        # sigmoid(logits), computed on host

        # ---------------- phase 1: factorized soft match ----------------
        B_ps = psB.tile([128, 3 * NCC], f32, tag="B", name="B")
        halves = [(0, h0)] + ([(h0, h1)] if h1 else [])
        with tc.tile_pool(name="psW", bufs=2,
                          space=bass.MemorySpace.PSUM) as psW:
            wx_list = []
            for hi_, (c0, hn) in enumerate(halves):
                vt_h = vt_t0 if hi_ == 0 else vt_t1
                ct = psW.tile([128, hn * NL], f32, tag=f"cross{hi_}",
                              name=f"cross{hi_}")
                wx = work.tile([128, hn * NL], f16, tag=f"wx{hi_}",
                               name=f"wx{hi_}")
                for j in range(hn):
                    nc.tensor.matmul(ct[:, j * NL:(j + 1) * NL],
                                     vt_h[:, j * 128:(j + 1) * 128],
                                     rhs_t[:], start=True, stop=True)
                nc.scalar.activation(wx[:], ct[:], AF.Exp,
                                     bias=0.0, scale=EXP_SCALE)
                wx_list.append(wx)
            for hi_, (c0, hn) in enumerate(halves):
                wx = wx_list[hi_]
                for j in range(hn):
                    c = c0 + j
                    wze = work.tile([128, 3 * NCC], f16, tag="wze",
                                    name=f"wze{c}")
                    e3b = e3_t[:, 3 * c:3 * c + 3] \
                        .rearrange("p (one f) -> p one f", one=1) \
                        .broadcast_to([128, NCC, 3])
                    wzb = wx[:, j * NL + NXY:(j + 1) * NL] \
                        .rearrange("p (k one) -> p k one", one=1) \
                        .broadcast_to([128, NCC, 3])
                    nc.vector.tensor_tensor(
                        wze[:].rearrange("p (k f) -> p k f", f=3),
                        e3b, wzb, ALU.mult)
                    nc.tensor.matmul(B_ps[:], wx[:, j * NL:j * NL + NXY],
                                     wze[:], start=(c == 0),
                                     stop=(c == nch - 1))

        # dummy Sin reading the last phase-1 exp output: anchors the
        # sin-table load right after the phase-1 exps in the ACT stream,
        # so it runs during the B-accumulate window.
        wx_last = wx_list[-1]
        lo = (halves[-1][1] - 1) * NL
        nc.scalar.activation(scr[:], wx_last[0:1, lo:lo + 1], AF.Sin)

        bs3 = B_ps[:].rearrange("p (k f) -> p k f", f=3)

        with tc.tile_pool(name="psM", bufs=1,
                          space=bass.MemorySpace.PSUM) as psM:
            def pt(tag):
                return parm.tile([128, NCC], f32, tag=tag, name=tag)

            # ---------------- per-contact params ----------------
            # phos_size == 1 always (max KSIG/|m| = 0.46 < 1 for ecc>=0),
            # so sr == 1 and the whole magnification chain drops out.
            t0 = pt("t0")
            nc.vector.tensor_scalar_add(t0[:], bs3[:, :, 2], 1e-8)
            rws = pt("rws"); nc.vector.reciprocal(rws[:], t0[:])
            # pe2 = [pol | ecc] in one op
            pe2 = parm.tile([128, 2 * NCC], f32, tag="pe2", name="pe2")
            rwsb = rws[:].rearrange("p (one k) -> p one k", one=1) \
                .broadcast_to([128, 2, NCC])
            bpol = B_ps[:].rearrange("p (k f) -> p f k", f=3)[:, 0:2, :]
            nc.vector.tensor_tensor(
                pe2[:].rearrange("p (f k) -> p f k", f=2),
                bpol, rwsb, ALU.mult)
            pol = pe2[:, 0:NCC]
            ecc = pe2[:, NCC:2 * NCC]

            # t20 = [pi/2 - |theta| | theta]; ACT Sin gives [cos | sin].
            t20 = parm.tile([128, 2 * NCC], f32, tag="t20", name="t20")
            nc.vector.tensor_scalar(t20[:, NCC:2 * NCC], pol, _DEG2RAD, -PI,
                                    ALU.mult, ALU.add)
            nc.vector.tensor_scalar(t20[:, NCC:2 * NCC], t20[:, NCC:2 * NCC],
                                    PI, -PI, ALU.min, ALU.max)
            ya = pt("ya")
            nc.scalar.activation(ya[:], t20[:, NCC:2 * NCC], AF.Abs)
            nc.vector.tensor_scalar(t20[:, 0:NCC], ya[:], -1.0, PI / 2.0,
                                    ALU.mult, ALU.add)
            sc20 = parm.tile([128, 2 * NCC], f32, tag="sc20", name="sc20")
            nc.scalar.activation(sc20[:], t20[:], AF.Sin)
            # dummy exp reading the Sin output: anchors the exp-table
            # reload right after the Sin, ahead of the phase-2 exps.
            nc.scalar.activation(scr[:], sc20[0:1, 0:1], AF.Exp,
                                 bias=0.0, scale=1.0)
            cs = sc20[:, 0:NCC]
            sn = sc20[:, NCC:2 * NCC]

            # t12 = [ecc*cos | ecc*sin] in one op
            t12 = parm.tile([128, 2 * NCC], f32, tag="t12", name="t12")
            eccb = pe2[:].rearrange("p (f k) -> p f k", f=2)[:, 1:2, :] \
                .broadcast_to([128, 2, NCC])
            nc.vector.tensor_tensor(
                t12[:].rearrange("p (f k) -> p f k", f=2),
                eccb, sc20[:].rearrange("p (f k) -> p f k", f=2), ALU.mult)
            sby = pt("sby")
            nc.vector.tensor_scalar(sby[:], t12[:, 0:NCC], -SE,
                                    -127.0 + WOFF, ALU.mult, ALU.add)
            sbx = pt("sbx")
            nc.vector.tensor_scalar(sbx[:], t12[:, NCC:2 * NCC], SE,
                                    -128.0 + WOFF, ALU.mult, ALU.add)

            # val/wc only feed the yy stage -- emit after the sby/sbx
            # critical chain so the scheduler keeps them off it
            val = pt("val")
            nc.vector.tensor_scalar_min(val[:], bs3[:, :, 2], 1.0)
            wc = parm.tile([128, NCC], f16, tag="wc", name="wc")
            nc.vector.tensor_mul(wc[:], pb[:], val[:])



            # ---------------- phase 2: separable splat ----------------
            # All factors computed on the 128-wide center window.
            # Squares split: ACT Square (bias per partition) for batch-0
            # sides and cols {2,3}; DVE (xs,sq) pair-ops for the rest.
            # GpSimd does NO phase-2 elementwise: concurrent DVE+GpSimd
            # SBUF traffic slows DVE ~3.5x (measured).
            mp = psM.tile([WIN, WIN], f32, tag="map", name="map")
            BATCHES = [(0, 2), (2, 4), (6, 4)]
            ACT_ROWS = {0, 1}
            ACT_COLS = {0, 1, 2, 3, 8, 9}
            sq_tiles = {}
            for b0, BN in BATCHES:
                W = BN * WIN
                sqb = work.tile([128, 2 * W], f16, tag="sqb",
                                name=f"sqb{b0}")
                sq_tiles[b0] = sqb

                def emit_sides(base, sbv, members):
                    q = 0
                    while q < BN:
                        c = b0 + q
                        if c in members:
                            nc.scalar.activation(
                                sqb[:, base + q * WIN:base + (q + 1) * WIN],
                                iof[:], AF.Square, bias=sbv[:, c:c + 1],
                                scale=1.0)
                            q += 1
                        else:
                            q1 = q
                            while q1 < BN and (b0 + q1) not in members:
                                q1 += 1
                            gn = q1 - q
                            xsp = work.tile([128, gn * WIN], f16,
                                            tag="xsp",
                                            name=f"xsp{base}_{b0}_{q}")
                            iofp = iof[:].rearrange(
                                "p (one n) -> p one n", one=1) \
                                .broadcast_to([128, gn, WIN])
                            sbp = sbv[:, b0 + q:b0 + q + gn] \
                                .rearrange("p (k one) -> p k one", one=1) \
                                .broadcast_to([128, gn, WIN])
                            nc.vector.tensor_tensor(
                                xsp[:].rearrange("p (k n) -> p k n",
                                                 n=WIN),
                                iofp, sbp, ALU.add)
                            nc.vector.tensor_tensor(
                                sqb[:, base + q * WIN:
                                    base + (q + gn) * WIN],
                                xsp[:], xsp[:], ALU.mult)
                            q = q1
                emit_sides(0, sby, ACT_ROWS)
                emit_sides(W, sbx, ACT_COLS)
            for b0, BN in BATCHES:
                W = BN * WIN
                sqb = sq_tiles[b0]
                xy8 = work.tile([128, 2 * W], f16, tag="xy8",
                                name=f"xy8{b0}")
                nc.scalar.activation(xy8[:], sqb[:], AF.Exp,
                                     bias=0.0, scale=-1.0)
                yyb = work.tile([128, W], f16, tag="yyb", name=f"yyb{b0}")
                wcb = wc[:, b0:b0 + BN] \
                    .rearrange("p (k one) -> p k one", one=1) \
                    .broadcast_to([128, BN, WIN])
                # high priority: once the batch's exp lands, the yy op
                # preempts remaining square work on DVE so the matmuls
                # can start
                with tc.high_priority():
                    nc.vector.tensor_tensor(
                        yyb[:].rearrange("p (k n) -> p k n", n=WIN),
                        xy8[:, 0:W].rearrange("p (k n) -> p k n", n=WIN),
                        wcb, ALU.mult)
                for q in range(BN):
                    c = b0 + q
                    yy = yyb[:, q * WIN:(q + 1) * WIN]
                    xx = xy8[:, W + q * WIN:W + (q + 1) * WIN]
                    nc.tensor.matmul(mp[:], yy, xx,
                                     start=(c == 0), stop=(c == NCC - 1))

            # ---------------- normalize + store ----------------
            # mx in f16: the max of f16-rounded row-maxes feeds an exact
            # f16 transpose (eye is 0/1) and an exact f16 broadcast
            # matmul -- +-2.4e-4 on the normalization only.
            mx = parm.tile([WIN, 1], f16, tag="mx", name="mx")
            nc.vector.reduce_max(mx[:], mp[:], axis=mybir.AxisListType.X)
            mt = psM.tile([1, WIN], f16, tag="mt", name="mt")
            nc.tensor.transpose(mt[:], mx[:], eye_t[0:WIN, 0:WIN])
            gm = parm.tile([1, 1], f16, tag="gm", name="gm")
            nc.vector.reduce_max(gm[:], mt[:], axis=mybir.AxisListType.X)
            gb = psM.tile([WIN, 1], f32, tag="gb", name="gb")
            nc.tensor.matmul(gb[:], ones16[:, 0:WIN], gm[:],
                             start=True, stop=True)
            ge = parm.tile([WIN, 1], f32, tag="ge", name="ge")
            nc.vector.tensor_scalar_add(ge[:], gb[:], 1e-8)
            gs = parm.tile([WIN, 1], f32, tag="gs", name="gs")
            nc.vector.reciprocal(gs[:], ge[:])

            o0 = work.tile([WIN, WIN], f32, tag="o0", name="o0")
            nc.vector.tensor_scalar_mul(o0[0:64, :], mp[0:64, :],
                                        gs[0:64, :])
            nc.sync.dma_start(out_d[WOFF:WOFF + 64, WOFF:WOFF + WIN],
                              o0[0:64, :])
            nc.scalar.activation(o0[64:WIN, :], mp[64:WIN, :], AF.Copy,
                                 scale=gs[64:WIN, :])
            nc.scalar.dma_start(
                out_d[WOFF + 64:WOFF + WIN, WOFF:WOFF + WIN],
                o0[64:WIN, :])
    return nc


# ----------------------------------------------------------------- entry
def _run(inputs, trace=False):
    params = np.asarray(inputs["params"], np.float32)
    logits = np.asarray(inputs["electrode_logits"], np.float32)
    v1_pos = np.asarray(inputs["v1_pos"], np.float32)
    v1_prf = np.asarray(inputs["v1_prf"], np.float32)
    start_loc = np.asarray(inputs["start_loc"], np.float32)
    surf_dist_lut = np.asarray(inputs["surf_dist_lut"], np.float32)
    alpha_grid = np.asarray(inputs["alpha_grid"], np.float32)
    beta_grid = np.asarray(inputs["beta_grid"], np.float32)

    gc, R, direction, shank = _host_geometry(
        params, start_loc, surf_dist_lut, alpha_grid, beta_grid)
    keeps = [_voxel_keep(v1_pos, gc[b], R[b, :, 2], shank[b] / 2.0)
             for b in range(B)]
    nkeep = max(int(k.sum()) for k in keeps)
    VP = max(256, ((nkeep + 127) // 128) * 128)
    nch = VP // 128
    VP0 = ((nch + 1) // 2) * 128

    in_maps = []
    for b in range(B):
        k = keeps[b]
        in_maps.append(_prep_core(gc[b], R[b], shank[b], logits[b],
                                  v1_pos[k], v1_prf[k], VP, VP0))
    nc = _build_nc(VP)
    _split_multiwaits(nc)
    res = run_bass_kernel_spmd(nc, in_maps, list(range(B)), trace=trace)
    out = np.stack([res.results[i]["out"] for i in range(B)])
    return out[:, None, :, :].astype(np.float32), res


def kernel(**inputs) -> np.ndarray:
    out, _ = _run(inputs, trace=False)
    return out


# revision 49
# speedup vs baseline: 1.1575x; 1.0186x over previous
"""Trainium2 Bass kernel for nn_DifferentiableSimulator.

Strategy (8 NeuronCores, B=8): one batch element per core, no collectives.

Host side (cheap, O(V+N)):
  - per-batch probe geometry: rotation, LUT bilinear interp (tiny)
  - per-batch voxel relevance sharding: keep voxels within CUT(7.5mm) +
    probe-radius of the shank axis segment.  Dropped voxels have weights
    < e^-14 relative to any weight that can influence an output pixel;
    empirically the output matches the dense reference to ~1e-3.
  - lattice factorization: the 1000 contacts are a rigid 10x10x10 grid,
    so in the rotated frame  d2[n,v] = (x_i-wx_v)^2 + (y_j-wy_v)^2 +
    (z_k-wz_v)^2  with w = R^T (v - grid_center).  The soft-match weight
    matrix factorizes as W[n,v] = Wxy[(ij),v] * Wz[k,v]: only 110 gaussian
    columns per voxel instead of 1000.  Host ships the voxel features
    (fp16 hi/lo pairs so the fp16 matmul is ~fp32-exact: fp16 products are
    exact in the fp32 PSUM accumulator) and the 138 lattice columns.
  - contacts are reindexed m = k*128 + (iy*10+ix)  (28 dummy xy slots per
    z-layer with weight 0) so the per-z-layer weighted sums land exactly
    in contact-chunk layout with no transposes.

Device side (per core), phase 1 -- soft PRF match, halves of the voxel
chunks: cross matmuls for a half land in one PSUM tile, ONE mega-exp
converts the half to fp16 gaussian weights, then per chunk a DVE op
forms WzE (fp16) and a single-pass fp16 matmul accumulates
B[128ij, 30] = sum_v Wxy^T (Wz*E) in fp32 PSUM.

Phase 2 -- separable splat (phos_sigma*SE < 0.46 for every reachable
ecc >= 0, so the max(.,1) clamp makes every phosphene sigma exactly
1 px; the 1/s scale drops out).  Per-contact centers via the hardware
Sin spline (sin table preloaded by a dummy op during phase 1, exp
table reloaded by a dummy right after).  Row/col gaussian arguments are
computed in TWO mega DVE/GpSimd ops per batch (broadcast APs over the
chunk axis amortize the ~160ns DVE instruction overhead), one mega-exp
per batch, then 20 fp16 matmuls accumulate the 256x256 map.

Normalize: row maxes on DVE, cross-partition max via the GpSimd
partition_all_reduce, reciprocal per partition, scale on DVE+ACT,
DMA out on two queues.
"""
import math
from contextlib import ExitStack

import numpy as np

import concourse.bass as bass
import concourse.bass_isa as bass_isa
import concourse.mybir as mybir
from concourse import tile
from concourse.bass_utils import run_bass_kernel_spmd

# ---- constants (must match the reference) ----
_CMAG_A = 0.75
_CMAG_B = 120.0
_CMAG_K = 17.3
_DEG2RAD = math.pi / 180.0
AMP = 100.0
_SPREAD = math.sqrt(AMP / 675.0)
VIEW_ANGLE = 90.0
MAP_SIZE = 256
SOFT_MATCH_SIGMA = 1.5

B = 8
NCC = 10                  # contact chunks = z-layers
NXY = 128                 # xy-lattice slots per layer (100 real + 28 dummy)
CUT = 7.5
XY_RAD = 1.8 * math.sqrt(2.0)
SE = MAP_SIZE / VIEW_ANGLE
EXP_SCALE = 2.0 / (2.0 * SOFT_MATCH_SIGMA ** 2)   # 2/4.5
NL = NXY + 10             # 138 lattice columns

f32 = mybir.dt.float32
f16 = mybir.dt.float16
i32 = mybir.dt.int32
AF = mybir.ActivationFunctionType
ALU = mybir.AluOpType
PI = math.pi


# ---------------------------------------------------------------- host prep
def _f16s(x):
    hi = np.float16(x)
    lo = np.float16(np.float32(x) - np.float32(hi))
    return hi, lo


def _f16_split(x):
    hi = x.astype(np.float16)
    lo = (x.astype(np.float32) - hi.astype(np.float32)).astype(np.float16)
    return hi.astype(np.float32), lo.astype(np.float32)


def _host_geometry(params, start_loc, surf_dist_lut, alpha_grid, beta_grid):
    params = params.astype(np.float64)
    alpha, beta, offset, shank = (params[:, 0], params[:, 1],
                                  params[:, 2], params[:, 3])
    a = alpha * _DEG2RAD
    b = beta * _DEG2RAD
    ca, sa = np.cos(a), np.sin(a)
    cb, sb = np.cos(b), np.sin(b)
    Bn = params.shape[0]
    Rx = np.zeros((Bn, 3, 3)); Ry = np.zeros((Bn, 3, 3))
    Rx[:, 0, 0] = 1; Rx[:, 1, 1] = ca; Rx[:, 1, 2] = -sa
    Rx[:, 2, 1] = sa; Rx[:, 2, 2] = ca
    Ry[:, 0, 0] = cb; Ry[:, 0, 2] = sb; Ry[:, 1, 1] = 1
    Ry[:, 2, 0] = -sb; Ry[:, 2, 2] = cb
    R = Rx @ Ry
    direction = np.einsum('bij,j->bi', R, np.array([0.0, 0.0, -1.0]))
    direction = direction / np.linalg.norm(direction, axis=-1, keepdims=True)
    lut = surf_dist_lut.astype(np.float64)
    na, nb = lut.shape
    ag, bg = alpha_grid.astype(np.float64), beta_grid.astype(np.float64)
    a_norm = 2.0 * (alpha - ag[0]) / (ag[-1] - ag[0] + 1e-08) - 1.0
    b_norm = 2.0 * (beta - bg[0]) / (bg[-1] - bg[0] + 1e-08) - 1.0
    ai = np.clip((a_norm + 1.0) * 0.5 * (na - 1), 0.0, na - 1.0)
    bi = np.clip((b_norm + 1.0) * 0.5 * (nb - 1), 0.0, nb - 1.0)
    a0 = np.clip(np.floor(ai), 0, na - 1).astype(np.int64)
    b0 = np.clip(np.floor(bi), 0, nb - 1).astype(np.int64)
    a1 = np.minimum(a0 + 1, na - 1)
    b1 = np.minimum(b0 + 1, nb - 1)
    fa = ai - a0
    fb = bi - b0
    v00 = lut[a0, b0]; v01 = lut[a0, b1]; v10 = lut[a1, b0]; v11 = lut[a1, b1]
    surf = (v00 * (1 - fa) * (1 - fb) + v01 * (1 - fa) * fb
            + v10 * fa * (1 - fb) + v11 * fa * fb)
    surf = np.maximum(surf, 1.0)
    penetration = surf - shank / 2.0 - offset
    grid_center = (start_loc.astype(np.float64)[None, :]
                   + direction * penetration[:, None])
    return grid_center, R, direction, shank


def _voxel_keep(v1_pos, grid_center, axis_dir, half_len):
    d = v1_pos.astype(np.float64) - grid_center[None, :]
    t = np.clip(d @ axis_dir, -half_len, half_len)
    dist = np.linalg.norm(d - t[:, None] * axis_dir[None, :], axis=1)
    return dist <= (CUT + XY_RAD + 0.5)


def _prep_core(gc_b, R_b, shank_b, logits_b, v1_pos_k, v1_prf_k, VP, VP0):
    """Per-core device input arrays for the lattice-factorized kernel."""
    Vk = v1_pos_k.shape[0]
    w = np.zeros((VP, 3))
    w[:Vk] = (v1_pos_k.astype(np.float64) - gc_b[None, :]) @ R_b
    wf = w.astype(np.float32)
    wh, wl = _f16_split(wf)
    bxy = (-0.5 * (w[:, 0] ** 2 + w[:, 1] ** 2)).astype(np.float32)
    bz = (-0.5 * w[:, 2] ** 2).astype(np.float32)
    bxy[Vk:] = -30000.0
    bz[Vk:] = -30000.0
    bxyh, bxyl = _f16_split(bxy)
    bzh, bzl = _f16_split(bz)
    onesv = np.ones(VP, np.float32)
    vt = np.stack([wh[:, 0], wh[:, 1], wl[:, 0], wl[:, 1], wh[:, 0],
                   wh[:, 1], onesv, onesv, bxyh, bxyl,
                   wh[:, 2], wl[:, 2], wh[:, 2], onesv, onesv, bzh, bzl],
                  axis=0).astype(np.float16)

    xs = np.arange(10) * 0.4 - 1.8
    zs = (np.linspace(0.0, 1.0, 10) - 0.5) * float(shank_b)
    cols = np.zeros((17, NXY + 10), np.float32)
    for ij in range(NXY):
        if ij < 100:
            iy, ix = ij // 10, ij % 10
            x, y = xs[ix], xs[iy]
            xh, xl = _f16s(x)
            yh, yl = _f16s(y)
            axyh, axyl = _f16s(-0.5 * (x * x + y * y))
            cols[0:10, ij] = [xh, yh, xh, yh, xl, yl, axyh, axyl, 1.0, 1.0]
        else:
            cols[6, ij] = -30000.0     # dummy xy slot -> Wxy = 0
            cols[8, ij] = 1.0
    for k in range(10):
        z = zs[k]
        zh, zl = _f16s(z)
        azh, azl = _f16s(-0.5 * z * z)
        cols[10:17, NXY + k] = [zh, zh, zl, azh, azl, 1.0, 1.0]
    rhs = cols.astype(np.float16)

    nch = VP // 128
    e3 = np.zeros((VP, 3), np.float32)
    e3[:Vk, 0] = v1_prf_k[:, 0]
    e3[:Vk, 1] = v1_prf_k[:, 1]
    e3[:Vk, 2] = 1.0
    e3t = np.ascontiguousarray(
        e3.reshape(nch, 128, 3).transpose(1, 0, 2).reshape(128, 3 * nch))

    lgt = np.full((NXY, NCC), -30.0, np.float32)
    iy, ix = np.divmod(np.arange(100), 10)
    for k in range(NCC):
        lgt[:100, k] = logits_b[iy * 100 + ix * 10 + k]
    lgt = 1.0 / (1.0 + np.exp(-lgt.astype(np.float64)))   # sigmoid on host
    vtc = np.ascontiguousarray(vt)
    return {"vt0": np.ascontiguousarray(vtc[:, :VP0]),
            "vt1": np.ascontiguousarray(vtc[:, VP0:]),
            "rhs": rhs, "e3": e3t,
            "lgt": np.ascontiguousarray(lgt.astype(np.float32)),
            "eye": np.eye(128, dtype=np.float16)}


# ------------------------------------------------------------- device kernel
def _split_multiwaits(nc):
    """This walrus build accepts at most ONE sync wait per instruction.
    Tile emits several.  Engine instruction streams execute in order, so
    moving all but one wait onto single-wait NoOps inserted just before
    the instruction preserves semantics exactly."""
    cnt = 0
    for fn in nc.m.functions:
        for blk in fn.blocks:
            out = []
            for inst in blk.instructions:
                si = inst.sync_info
                if si is not None and si.on_wait is not None \
                        and len(si.on_wait) > 1:
                    waits = list(si.on_wait)
                    for w in waits[:-1]:
                        cnt += 1
                        out.append(mybir.InstNoOp(
                            name=f"WSPLIT-{cnt}",
                            engine=inst.engine,
                            ins=[], outs=[],
                            sync_info=mybir.SyncInfo(on_wait=[w],
                                                     on_update=[]),
                        ))
                    inst.sync_info = mybir.SyncInfo(
                        on_wait=[waits[-1]], on_update=list(si.on_update))
                out.append(inst)
            blk.instructions = out
    return cnt


def _build_nc(VP):
    nch = VP // 128
    h0 = (nch + 1) // 2          # chunks in first half
    h1 = nch - h0
    VP0 = h0 * 128
    nc = bass.Bass()
    vt0_d = nc.dram_tensor("vt0", [17, VP0], f16, kind="ExternalInput")
    vt1_d = (nc.dram_tensor("vt1", [17, VP - VP0], f16, kind="ExternalInput")
             if h1 else None)
    rhs_d = nc.dram_tensor("rhs", [17, NL], f16, kind="ExternalInput")
    e3_d = nc.dram_tensor("e3", [128, 3 * nch], f32, kind="ExternalInput")
    lgt_d = nc.dram_tensor("lgt", [NXY, NCC], f32, kind="ExternalInput")
    eye_d = nc.dram_tensor("eye", [128, 128], f16, kind="ExternalInput")
    out_d = nc.dram_tensor("out", [MAP_SIZE, MAP_SIZE], f32,
                           kind="ExternalOutput")

    with ExitStack() as ctx:
        tc = ctx.enter_context(tile.TileContext(nc))
        constp = ctx.enter_context(tc.tile_pool(name="const", bufs=1))
        parm = ctx.enter_context(tc.tile_pool(name="parm", bufs=1))
        work = ctx.enter_context(tc.tile_pool(name="work", bufs=6))
        psB = ctx.enter_context(
            tc.tile_pool(name="psB", bufs=1, space=bass.MemorySpace.PSUM))

        # Warmups first (top scheduler priority): ACT table load + PE HAM
        # burst run during the sem-init + input-DMA window.
        scr = constp.tile([1, 1], f32, tag="scr", name="scr")
        nc.vector.memset(scr[:], 0.0)
        nc.scalar.activation(scr[:], scr[:], AF.Exp, bias=0.0, scale=1.0)
        wrm = constp.tile([128, 256], f16, tag="wrm", name="wrm")
        nc.vector.memset(wrm[:], 0.0)
        with tc.tile_pool(name="psWp", bufs=1,
                          space=bass.MemorySpace.PSUM) as psWp:
            wps = psWp.tile([128, 256], f32, tag="wps", name="wps")
            for _ in range(12):
                nc.tensor.matmul(wps[:], wrm[:, 0:128], wrm[:],
                                 start=True, stop=True, skip_group_check=True)

        # ---------------- input DMAs (4 queues) ----------------
        rhs_t = constp.tile([17, NL], f16, tag="rhs", name="rhs")
        nc.sync.dma_start(rhs_t[:], rhs_d[:])
        vt_t0 = constp.tile([17, VP0], f16, tag="vt0", name="vt0")
        nc.sync.dma_start(vt_t0[:], vt0_d[:])
        e3_t = constp.tile([128, 3 * nch], f32, tag="e3", name="e3")
        with tc.high_priority():
            nc.gpsimd.dma_start(e3_t[:], e3_d[:])
        if h1:
            vt_t1 = constp.tile([17, VP - VP0], f16, tag="vt1", name="vt1")
            nc.scalar.dma_start(vt_t1[:], vt1_d[:])
        eye_t = constp.tile([128, 128], f16, tag="eye", name="eye")
        nc.gpsimd.dma_start(eye_t[:], eye_d[:])
        ones16 = constp.tile([1, 128], f16, tag="ones16", name="ones16")
        nc.vector.memset(ones16[:], 1.0)
        lg_t = constp.tile([NXY, NCC], f32, tag="lgt", name="lgt")
        nc.scalar.dma_start(lg_t[:], lgt_d[:])

        # Window: every phosphene center is within |c-128| <= 12*SE+eps
        # = 34.2 px and sigma == 1 px, so the map is (sub-1e-6) zero
        # outside the centered 128x128 window [64,192).  Compute factors,
        # matmuls, and normalization on the window only; pre-write the
        # zero border during the input-DMA dead time.
        WIN, WOFF = 96, 80
        ii_t = constp.tile([128, WIN], i32, tag="ii", name="ii")
        nc.gpsimd.iota(ii_t[:], pattern=[[1, WIN]], base=0,
                       channel_multiplier=0)
        iof = constp.tile([128, WIN], f32, tag="iof", name="iof")
        nc.vector.tensor_copy(iof[:], ii_t[:])

        zt = constp.tile([128, MAP_SIZE], f32, tag="zt", name="zt")
        nc.vector.memset(zt[:], 0.0)
        nc.sync.dma_start(out_d[0:WOFF, :], zt[0:WOFF, :])
        nc.sync.dma_start(out_d[WOFF + WIN:MAP_SIZE, :], zt[0:WOFF, :])
        nc.gpsimd.dma_start(out_d[WOFF:WOFF + WIN, 0:WOFF],
                            zt[0:WIN, 0:WOFF])
        nc.gpsimd.dma_start(out_d[WOFF:WOFF + WIN, WOFF + WIN:MAP_SIZE],
                            zt[0:WIN, 0:WOFF])

        pb = lg_t        # sigmoid(logits), computed on host

        # ---------------- phase 1: factorized soft match ----------------
        B_ps = psB.tile([128, 3 * NCC], f32, tag="B", name="B")
        halves = [(0, h0)] + ([(h0, h1)] if h1 else [])
        with tc.tile_pool(name="psW", bufs=2,
                          space=bass.MemorySpace.PSUM) as psW:
            wx_list = []
            for hi_, (c0, hn) in enumerate(halves):
                vt_h = vt_t0 if hi_ == 0 else vt_t1
                ct = psW.tile([128, hn * NL], f32, tag=f"cross{hi_}",
                              name=f"cross{hi_}")
                wx = work.tile([128, hn * NL], f16, tag=f"wx{hi_}",
                               name=f"wx{hi_}")
                for j in range(hn):
                    nc.tensor.matmul(ct[:, j * NL:(j + 1) * NL],
                                     vt_h[:, j * 128:(j + 1) * 128],
                                     rhs_t[:], start=True, stop=True)
                nc.scalar.activation(wx[:], ct[:], AF.Exp,
                                     bias=0.0, scale=EXP_SCALE)
                wx_list.append(wx)
            for hi_, (c0, hn) in enumerate(halves):
                wx = wx_list[hi_]
                for j in range(hn):
                    c = c0 + j
                    wze = work.tile([128, 3 * NCC], f16, tag="wze",
                                    name=f"wze{c}")
                    e3b = e3_t[:, 3 * c:3 * c + 3] \
                        .rearrange("p (one f) -> p one f", one=1) \
                        .broadcast_to([128, NCC, 3])
                    wzb = wx[:, j * NL + NXY:(j + 1) * NL] \
                        .rearrange("p (k one) -> p k one", one=1) \
                        .broadcast_to([128, NCC, 3])
                    nc.vector.tensor_tensor(
                        wze[:].rearrange("p (k f) -> p k f", f=3),
                        e3b, wzb, ALU.mult)
                    nc.tensor.matmul(B_ps[:], wx[:, j * NL:j * NL + NXY],
                                     wze[:], start=(c == 0),
                                     stop=(c == nch - 1))

        # dummy Sin reading the last phase-1 exp output: anchors the
        # sin-table load right after the phase-1 exps in the ACT stream,
        # so it runs during the B-accumulate window.
        wx_last = wx_list[-1]
        lo = (halves[-1][1] - 1) * NL
        nc.scalar.activation(scr[:], wx_last[0:1, lo:lo + 1], AF.Sin)

        bs3 = B_ps[:].rearrange("p (k f) -> p k f", f=3)

        with tc.tile_pool(name="psM", bufs=1,
                          space=bass.MemorySpace.PSUM) as psM:
            def pt(tag):
                return parm.tile([128, NCC], f32, tag=tag, name=tag)

            # ---------------- per-contact params ----------------
            # phos_size == 1 always (max KSIG/|m| = 0.46 < 1 for ecc>=0),
            # so sr == 1 and the whole magnification chain drops out.
            t0 = pt("t0")
            nc.vector.tensor_scalar_add(t0[:], bs3[:, :, 2], 1e-8)
            rws = pt("rws"); nc.vector.reciprocal(rws[:], t0[:])
            # pe2 = [pol | ecc] in one op
            pe2 = parm.tile([128, 2 * NCC], f32, tag="pe2", name="pe2")
            rwsb = rws[:].rearrange("p (one k) -> p one k", one=1) \
                .broadcast_to([128, 2, NCC])
            bpol = B_ps[:].rearrange("p (k f) -> p f k", f=3)[:, 0:2, :]
            nc.vector.tensor_tensor(
                pe2[:].rearrange("p (f k) -> p f k", f=2),
                bpol, rwsb, ALU.mult)
            pol = pe2[:, 0:NCC]
            ecc = pe2[:, NCC:2 * NCC]

            # t20 = [pi/2 - |theta| | theta]; ACT Sin gives [cos | sin].
            t20 = parm.tile([128, 2 * NCC], f32, tag="t20", name="t20")
            nc.vector.tensor_scalar(t20[:, NCC:2 * NCC], pol, _DEG2RAD, -PI,
                                    ALU.mult, ALU.add)
            nc.vector.tensor_scalar(t20[:, NCC:2 * NCC], t20[:, NCC:2 * NCC],
                                    PI, -PI, ALU.min, ALU.max)
            ya = pt("ya")
            nc.scalar.activation(ya[:], t20[:, NCC:2 * NCC], AF.Abs)
            nc.vector.tensor_scalar(t20[:, 0:NCC], ya[:], -1.0, PI / 2.0,
                                    ALU.mult, ALU.add)
            sc20 = parm.tile([128, 2 * NCC], f32, tag="sc20", name="sc20")
            nc.scalar.activation(sc20[:], t20[:], AF.Sin)
            # dummy exp reading the Sin output: anchors the exp-table
            # reload right after the Sin, ahead of the phase-2 exps.
            nc.scalar.activation(scr[:], sc20[0:1, 0:1], AF.Exp,
                                 bias=0.0, scale=1.0)
            cs = sc20[:, 0:NCC]
            sn = sc20[:, NCC:2 * NCC]

            # t12 = [ecc*cos | ecc*sin] in one op
            t12 = parm.tile([128, 2 * NCC], f32, tag="t12", name="t12")
            eccb = pe2[:].rearrange("p (f k) -> p f k", f=2)[:, 1:2, :] \
                .broadcast_to([128, 2, NCC])
            nc.vector.tensor_tensor(
                t12[:].rearrange("p (f k) -> p f k", f=2),
                eccb, sc20[:].rearrange("p (f k) -> p f k", f=2), ALU.mult)
            sby = pt("sby")
            nc.vector.tensor_scalar(sby[:], t12[:, 0:NCC], -SE,
                                    -127.0 + WOFF, ALU.mult, ALU.add)
            sbx = pt("sbx")
            nc.vector.tensor_scalar(sbx[:], t12[:, NCC:2 * NCC], SE,
                                    -128.0 + WOFF, ALU.mult, ALU.add)

            # val/wc only feed the yy stage -- emit after the sby/sbx
            # critical chain so the scheduler keeps them off it
            val = pt("val")
            nc.vector.tensor_scalar_min(val[:], bs3[:, :, 2], 1.0)
            wc = parm.tile([128, NCC], f16, tag="wc", name="wc")
            nc.vector.tensor_mul(wc[:], pb[:], val[:])



            # ---------------- phase 2: separable splat ----------------
            # All factors computed on the 128-wide center window.
            # Squares split: ACT Square (bias per partition) for batch-0
            # sides and cols {2,3}; DVE (xs,sq) pair-ops for the rest.
            # GpSimd does NO phase-2 elementwise: concurrent DVE+GpSimd
            # SBUF traffic slows DVE ~3.5x (measured).
            mp = psM.tile([WIN, WIN], f32, tag="map", name="map")
            BATCHES = [(0, 2), (2, 4), (6, 4)]
            ACT_ROWS = {0, 1}
            ACT_COLS = {0, 1, 2, 3, 8, 9}
            sq_tiles = {}
            for b0, BN in BATCHES:
                W = BN * WIN
                sqb = work.tile([128, 2 * W], f16, tag="sqb",
                                name=f"sqb{b0}")
                sq_tiles[b0] = sqb

                def emit_sides(base, sbv, members):
                    q = 0
                    while q < BN:
                        c = b0 + q
                        if c in members:
                            nc.scalar.activation(
                                sqb[:, base + q * WIN:base + (q + 1) * WIN],
                                iof[:], AF.Square, bias=sbv[:, c:c + 1],
                                scale=1.0)
                            q += 1
                        else:
                            q1 = q
                            while q1 < BN and (b0 + q1) not in members:
                                q1 += 1
                            gn = q1 - q
                            xsp = work.tile([128, gn * WIN], f16,
                                            tag="xsp",
                                            name=f"xsp{base}_{b0}_{q}")
                            iofp = iof[:].rearrange(
                                "p (one n) -> p one n", one=1) \
                                .broadcast_to([128, gn, WIN])
                            sbp = sbv[:, b0 + q:b0 + q + gn] \
                                .rearrange("p (k one) -> p k one", one=1) \
                                .broadcast_to([128, gn, WIN])
                            nc.vector.tensor_tensor(
                                xsp[:].rearrange("p (k n) -> p k n",
                                                 n=WIN),
                                iofp, sbp, ALU.add)
                            nc.vector.tensor_tensor(
                                sqb[:, base + q * WIN:
                                    base + (q + gn) * WIN],
                                xsp[:], xsp[:], ALU.mult)
                            q = q1
                emit_sides(0, sby, ACT_ROWS)
                emit_sides(W, sbx, ACT_COLS)
            for b0, BN in BATCHES:
                W = BN * WIN
                sqb = sq_tiles[b0]
                xy8 = work.tile([128, 2 * W], f16, tag="xy8",
                                name=f"xy8{b0}")
                nc.scalar.activation(xy8[:], sqb[:], AF.Exp,
                                     bias=0.0, scale=-1.0)
                yyb = work.tile([128, W], f16, tag="yyb", name=f"yyb{b0}")
                wcb = wc[:, b0:b0 + BN] \
                    .rearrange("p (k one) -> p k one", one=1) \
                    .broadcast_to([128, BN, WIN])
                # high priority: once the batch's exp lands, the yy op
                # preempts remaining square work on DVE so the matmuls
                # can start
                with tc.high_priority():
                    nc.vector.tensor_tensor(
                        yyb[:].rearrange("p (k n) -> p k n", n=WIN),
                        xy8[:, 0:W].rearrange("p (k n) -> p k n", n=WIN),
                        wcb, ALU.mult)
                for q in range(BN):
                    c = b0 + q
                    yy = yyb[:, q * WIN:(q + 1) * WIN]
                    xx = xy8[:, W + q * WIN:W + (q + 1) * WIN]
                    nc.tensor.matmul(mp[:], yy, xx,
                                     start=(c == 0), stop=(c == NCC - 1))

            # ---------------- normalize + store ----------------
            # mx in f16: the max of f16-rounded row-maxes feeds an exact
            # f16 transpose (eye is 0/1) and an exact f16 broadcast
            # matmul -- +-2.4e-4 on the normalization only.
            mx = parm.tile([WIN, 1], f16, tag="mx", name="mx")
            nc.vector.reduce_max(mx[:], mp[:], axis=mybir.AxisListType.X)
            mt = psM.tile([1, WIN], f16, tag="mt", name="mt")
            nc.tensor.transpose(mt[:], mx[:], eye_t[0:WIN, 0:WIN])
            gm = parm.tile([1, 1], f16, tag="gm", name="gm")
            nc.vector.reduce_max(gm[:], mt[:], axis=mybir.AxisListType.X)
            gb = psM.tile([WIN, 1], f32, tag="gb", name="gb")
            nc.tensor.matmul(gb[:], ones16[:, 0:WIN], gm[:],
                             start=True, stop=True)
            ge = parm.tile([WIN, 1], f32, tag="ge", name="ge")
            nc.vector.tensor_scalar_add(ge[:], gb[:], 1e-8)
            gs = parm.tile([WIN, 1], f32, tag="gs", name="gs")
            nc.vector.reciprocal(gs[:], ge[:])

            o0 = work.tile([WIN, WIN], f32, tag="o0", name="o0")
            nc.vector.tensor_scalar_mul(o0[0:64, :], mp[0:64, :],
                                        gs[0:64, :])
            nc.sync.dma_start(out_d[WOFF:WOFF + 64, WOFF:WOFF + WIN],
                              o0[0:64, :])
            nc.scalar.activation(o0[64:WIN, :], mp[64:WIN, :], AF.Copy,
                                 scale=gs[64:WIN, :])
            nc.scalar.dma_start(
                out_d[WOFF + 64:WOFF + WIN, WOFF:WOFF + WIN],
                o0[64:WIN, :])
    return nc


# ----------------------------------------------------------------- entry
def _run(inputs, trace=False):
    params = np.asarray(inputs["params"], np.float32)
    logits = np.asarray(inputs["electrode_logits"], np.float32)
    v1_pos = np.asarray(inputs["v1_pos"], np.float32)
    v1_prf = np.asarray(inputs["v1_prf"], np.float32)
    start_loc = np.asarray(inputs["start_loc"], np.float32)
    surf_dist_lut = np.asarray(inputs["surf_dist_lut"], np.float32)
    alpha_grid = np.asarray(inputs["alpha_grid"], np.float32)
    beta_grid = np.asarray(inputs["beta_grid"], np.float32)

    gc, R, direction, shank = _host_geometry(
        params, start_loc, surf_dist_lut, alpha_grid, beta_grid)
    keeps = [_voxel_keep(v1_pos, gc[b], R[b, :, 2], shank[b] / 2.0)
             for b in range(B)]
    nkeep = max(int(k.sum()) for k in keeps)
    VP = max(256, ((nkeep + 127) // 128) * 128)
    nch = VP // 128
    VP0 = ((nch + 1) // 2) * 128

    in_maps = []
    for b in range(B):
        k = keeps[b]
        in_maps.append(_prep_core(gc[b], R[b], shank[b], logits[b],
                                  v1_pos[k], v1_prf[k], VP, VP0))
    nc = _build_nc(VP)
    _split_multiwaits(nc)
    res = run_bass_kernel_spmd(nc, in_maps, list(range(B)), trace=trace)
    out = np.stack([res.results[i]["out"] for i in range(B)])
    return out[:, None, :, :].astype(np.float32), res


def kernel(**inputs) -> np.ndarray:
    out, _ = _run(inputs, trace=False)
    return out
